# revision 1
# baseline (speedup 1.0000x reference)
"""Trainium2 Bass kernel for nn_AddSparseAndLowRankCorrectionFP32.

The module computes
    out = x @ W_inner^T + b + alpha * (A16 @ (B16 @ x) + x @ S^T)
with A/B/sparse_values passed through an fp16 round-trip and S the dense
scatter of the COO sparse correction.  Everything is linear in x, so the
whole module folds into a single dense matmul:
    W_eff = W_inner + A16 @ B16 + S        (folded on host)
    out   = x @ W_eff^T + b                (device)

Sharding: data-parallel over the 8192 tokens (1024 per core), W_eff and
bias replicated — the first option in the problem's sharding hint.  Each
core computes its output shard transposed ([d_out, tokens]) so that the
weight matrix is the PE-stationary operand and is streamed from HBM
exactly once while the x shard (16.8 MB) stays SBUF-resident.  Matmuls
run in float32r (fp32 storage, reduced-precision PE mode, 4x the fp32
matmul throughput, measured rel err ~1.5e-4); accumulation is fp32 in
PSUM.  Per core: 32 o_tiles x 32 k_tiles x 2 token-slices = 2048 matmuls
of [128x128]@[128x512], issued back-to-back (measured ~227 ns/MM warm);
PSUM double-buffered per o_tile, drained by the Scalar engine with a
fused per-partition bias add, written out by HW-DGE DMA.

Measured on the 8-core TRN2 (cool chip): ~532 us NEFF exec time
(~466 us PE-bound matmul stream + ~60 us HBM-bound input load + tail),
rel err 1.5e-4 vs the fp32 reference.
"""

import contextlib
import os

import ml_dtypes
import numpy as np

import concourse.bass as bass
import concourse.mybir as mybir
from concourse.bass_utils import run_bass_kernel_spmd

# "f32r": fp32 data, reduced-precision PE mode (rel err ~1.5e-4)
# "bf16": half the DMA traffic, rel err ~4e-3
MM_DTYPE = os.environ.get("MM_DTYPE", "f32r")

N_CORES = 8
D = 4096                 # d_in == d_out
B_SZ, S_SZ = 4, 2048     # x is [4, 2048, 4096]
TOKENS = B_SZ * S_SZ
T = TOKENS // N_CORES    # tokens per core (1024)
P = 128
KT = D // P              # 32 k-tiles (contraction)
OT = D // P              # 32 output-row tiles
NS = 512                 # moving free dim per matmul
NSL = T // NS            # 2 token slices per core
W_BUFS = 3               # weight strip buffers (double+ buffering)

f32 = mybir.dt.float32
f32r = mybir.dt.float32r

_cache: dict = {}


def _build_nc() -> bass.Bass:
    key = f"nc_{MM_DTYPE}_{os.environ.get('X_FLOW', 'blast')}"
    if key in _cache:
        return _cache[key]
    mm_dt = {"f32r": f32r, "bf16": mybir.dt.bfloat16}[MM_DTYPE]

    nc = bass.Bass()
    xT_ext = nc.declare_dram_parameter("xT", [D, T], mm_dt, isOutput=False)
    wT_ext = nc.declare_dram_parameter("wT", [D, D], mm_dt, isOutput=False)
    b_ext = nc.declare_dram_parameter("bias", [P, OT], f32, isOutput=False)
    out_ext = nc.declare_dram_parameter("out", [D, T], f32, isOutput=True)

    # wT[d, o] viewed as [p, k, i, m]: d = k*128+p, o = i*128+m
    wT_t = wT_ext.rearrange("(k p) (i m) -> p k i m", p=P, m=P)
    xT_t = xT_ext.rearrange("(k p) t -> p k t", p=P)

    with contextlib.ExitStack() as stack:
        ec = stack.enter_context
        x_sb = ec(nc.sbuf_tensor("x_sb", [P, KT, T], mm_dt))
        w_sb = [ec(nc.sbuf_tensor(f"w_sb{j}", [P, KT, P], mm_dt)) for j in range(W_BUFS)]
        b_sb = ec(nc.sbuf_tensor("b_sb", [P, OT], f32))
        o_sb = [ec(nc.sbuf_tensor(f"o_sb{j}", [P, T], f32)) for j in range(2)]
        ps = [ec(nc.psum_tensor(f"ps{j}", [P, T], f32)) for j in range(2)]
        in_sem = ec(nc.semaphore("in_sem"))
        w_sem = ec(nc.semaphore("w_sem"))
        pe_sem = ec(nc.semaphore("pe_sem"))
        act_sem = ec(nc.semaphore("act_sem"))
        od_sem = ec(nc.semaphore("od_sem"))
        X_CHUNKS = 16
        KC = KT // X_CHUNKS
        xs = [ec(nc.semaphore(f"xs{j}")) for j in range(X_CHUNKS)]
        block = ec(nc.Block())

        def x_chunk(eng, c):
            eng.dma_start(
                out=x_sb[:, c * KC : (c + 1) * KC, :],
                in_=xT_t[:, c * KC : (c + 1) * KC, :],
            ).then_inc(xs[c], 16)

        # Startup: W strip 0 first, then x in k-order chunks (per-chunk
        # sems), issued from BOTH the sync (HW-DGE) and gpsimd (SW-DGE)
        # engines — a single engine serializes ~1us of enqueue per dma_start
        # and tops out ~270 GB/s; dual-engine unthrottled ("blast", default)
        # reaches ~380 GB/s so the whole 18.9 MB startup lands in ~50us.
        # Measured rankings (cool chip): blast 532us < gate0 553us (holding
        # the odd stream for chunk 0 trickles o_tile 0 but the sparse MM
        # stream oscillates the HAM clock gate — net loss) and depth8 545us
        # (in-flight caps under-use bandwidth).  Strips 1-2 slot in midway;
        # they're only needed once o_tile 0 (gated on the full x) completes.
        x_flow = os.environ.get("X_FLOW", "blast")

        @block.gpsimd
        def _(gp):
            if x_flow == "gate0":
                gp.wait_ge(xs[0], 16)
                gp.wait_ge(w_sem, 16)
            for c in range(1, X_CHUNKS, 2):
                if x_flow == "depth8" and c >= 9:
                    gp.wait_ge(xs[c - 8], 16)
                x_chunk(gp, c)

        @block.sync
        def _(sync):
            sync.dma_start(out=b_sb[:], in_=b_ext[:]).then_inc(in_sem, 16)

            def w_strip(i, buf=None):
                sync.dma_start(
                    out=w_sb[buf if buf is not None else i % W_BUFS][:],
                    in_=wT_t[:, :, i, :],
                ).then_inc(w_sem, 16)

            w_strip(0)
            x_chunk(sync, 0)
            # strips 1-2 right behind chunk 0: issuing them mid-x-stream
            # leaves the PE waiting on them between o_tiles 1-2 (~10-20us of
            # measured idle); up front they land well before they're needed.
            w_strip(1)
            w_strip(2)
            for c in range(2, X_CHUNKS, 2):
                if x_flow == "depth8" and c >= 8:
                    sync.wait_ge(xs[c - 8], 16)
                x_chunk(sync, c)
            for i in range(OT):
                if i + W_BUFS < OT:
                    sync.wait_ge(pe_sem, i + 1)
                    sync.dma_start(
                        out=w_sb[(i + W_BUFS) % W_BUFS][:],
                        in_=wT_t[:, :, i + W_BUFS, :],
                    ).then_inc(w_sem, 16)
                sync.wait_ge(act_sem, i + 1)
                sync.dma_start(
                    out=out_ext[i * P : (i + 1) * P, :], in_=o_sb[i % 2][:]
                ).then_inc(od_sem, 16)
            sync.wait_ge(od_sem, OT * 16)

        @block.tensor
        def _(pe):
            for i in range(OT):
                pe.wait_ge(w_sem, (i + 1) * 16)
                if i >= 2:
                    pe.wait_ge(act_sem, i - 1)
                for k in range(KT):
                    if i == 0 and k % KC == 0:
                        pe.wait_ge(xs[k // KC], 16)
                    last = k == KT - 1
                    w_ap = w_sb[i % W_BUFS][:, k, :]
                    for s in range(NSL):
                        mm = pe.matmul(
                            ps[i % 2][:, s * NS : (s + 1) * NS],
                            lhsT=w_ap,
                            rhs=x_sb[:, k, s * NS : (s + 1) * NS],
                            start=(k == 0),
                            stop=last,
                        )
                    if last:
                        mm.then_inc(pe_sem, 1)

        @block.scalar
        def _(act):
            act.wait_ge(in_sem, 16)  # bias loaded
            for i in range(OT):
                act.wait_ge(pe_sem, i + 1)
                if i >= 2:
                    act.wait_ge(od_sem, (i - 1) * 16)
                act.activation(
                    o_sb[i % 2][:],
                    ps[i % 2][:],
                    mybir.ActivationFunctionType.Identity,
                    bias=b_sb[:, i : i + 1],
                ).then_inc(act_sem, 1)

    _cache[key] = nc
    return nc


def _fold_weights(W_inner, A, B, sparse_values, sparse_indices):
    """W_eff = W_inner + fp16rt(A) @ fp16rt(B) + scatter(fp16rt(values))."""
    A16 = A.astype(np.float16).astype(np.float32)
    B16 = B.astype(np.float16).astype(np.float32)
    V16 = sparse_values.astype(np.float16).astype(np.float32)
    W = W_inner + A16 @ B16
    rows = np.asarray(sparse_indices[0], dtype=np.int64)
    cols = np.asarray(sparse_indices[1], dtype=np.int64)
    S = np.bincount(rows * D + cols, weights=V16, minlength=D * D)
    W += S.reshape(D, D).astype(np.float32)
    return W


def build_inmaps(inputs):
    x = np.asarray(inputs["x"], dtype=np.float32)
    W_inner = np.asarray(inputs["W_inner"], dtype=np.float32)
    b_inner = np.asarray(inputs["b_inner"], dtype=np.float32)
    A = np.asarray(inputs["A"], dtype=np.float32)
    B = np.asarray(inputs["B"], dtype=np.float32)
    sparse_values = np.asarray(inputs["sparse_values"], dtype=np.float32)
    sparse_indices = np.asarray(inputs["sparse_indices"])

    W = _fold_weights(W_inner, A, B, sparse_values, sparse_indices)
    mm_np = {"f32r": np.float32, "bf16": ml_dtypes.bfloat16}[MM_DTYPE]
    wT = np.ascontiguousarray(W.T.astype(mm_np))        # [d_in, d_out]
    biasT = np.ascontiguousarray(b_inner.reshape(OT, P).T)  # [128, OT]

    x2T = x.reshape(TOKENS, D).T.astype(mm_np)          # [d_in, tokens]
    in_maps = []
    for c in range(N_CORES):
        xT_c = np.ascontiguousarray(x2T[:, c * T : (c + 1) * T])
        in_maps.append({"xT": xT_c, "wT": wT, "bias": biasT})
    return in_maps


def run_device(in_maps, **kwargs):
    nc = _build_nc()
    return run_bass_kernel_spmd(nc, in_maps, core_ids=list(range(N_CORES)), **kwargs)


def postprocess(results, dtype=np.float32):
    out = np.empty((TOKENS, D), dtype=dtype)
    for c in range(N_CORES):
        out[c * T : (c + 1) * T, :] = results[c]["out"].T
    return out.reshape(B_SZ, S_SZ, D)


def kernel(**inputs) -> np.ndarray:
    in_maps = build_inmaps(inputs)
    res = run_device(in_maps)
    return postprocess(res.results, dtype=np.asarray(inputs["x"]).dtype)



# revision 8
# speedup vs baseline: 1.0863x; 1.0863x over previous
"""Trainium2 Bass kernel for nn_AddSparseAndLowRankCorrectionFP32.

The module computes
    out = x @ W_inner^T + b + alpha * (A16 @ (B16 @ x) + x @ S^T)
with A/B/sparse_values passed through an fp16 round-trip and S the dense
scatter of the COO sparse correction.  Everything is linear in x, so the
whole module folds into a single dense matmul:
    W_eff = W_inner + A16 @ B16 + S        (folded on host)
    out   = x @ W_eff^T + b                (device)

Sharding: data-parallel over the 8192 tokens (1024 per core), W_eff and
bias replicated.  Each core computes its output shard transposed
([d_out, tokens]) so the weight matrix is the PE-stationary operand.

Precision/throughput hybrid: the PE runs bf16 at 216 ns per
128x128@128x512 matmul and fp8e4m3 DoubleRow (contracting 256 rows) at
the same 216 ns — 2x the FLOP rate.  Pure fp8 misses the 2e-2 accuracy
gate (e4m3 quantization alone is ~2.6% per operand), but the error is
deterministic and scales as sqrt(fraction of K in fp8), so F k-plane
*pairs* (of 16) run as fp8 DoubleRow and the rest as bf16:
    F=0: rel err 2.3e-3   F=3: 1.64e-2   F=4: 1.89e-2  (gate: 2e-2)
Scales: x8 = e4m3(32x), W8 = e4m3(16W) -> fp8 partials carry 512x; the
bf16 weights are pre-scaled by 512 so every matmul accumulates at 512x
into the same PSUM chain, and the Scalar-engine drain applies
out = psum/512 + bias.

Schedule per core: o_tiles 0+1 run k-interleaved and chunk-gated so the
PE consumes the incoming x stream at ~2x the DMA arrival rate
(stall-free startup); o_tiles 2..31 run sequentially, PSUM
double-buffered, weight strips triple-buffered and prefetched.  PSUM is
drained per 512-token slice (fused bias + 1/512 rescale) which also
shrinks the end tail; outputs stream back via gpsimd-queue DMA.
"""

import contextlib
import os

import ml_dtypes
import numpy as np

import concourse.bass as bass
import concourse.mybir as mybir
from concourse.bass_utils import run_bass_kernel_spmd

N_CORES = 8
D = 4096                 # d_in == d_out
B_SZ, S_SZ = 4, 2048     # x is [4, 2048, 4096]
TOKENS = B_SZ * S_SZ
T = TOKENS // N_CORES    # tokens per core (1024)
P = 128
KT = D // P              # 32 k-planes total
OT = D // P              # 32 output-row tiles
NS = 512                 # PSUM-bank-limited moving dim per matmul
NSL = T // NS            # 2 token slices per core
W_BUFS = 3               # weight strip buffers

F = int(os.environ.get("F_PAIRS", "3"))  # fp8 DoubleRow k-plane pairs (0..16)
KB = KT - 2 * F          # bf16 k-planes
SX, SW = 32.0, 16.0      # fp8 scales; product 512 also applied to bf16 W
PSCALE = SX * SW

f32 = mybir.dt.float32
bf16 = mybir.dt.bfloat16
f8 = mybir.dt.float8e4
DR = mybir.MatmulPerfMode.DoubleRow

_cache: dict = {}


def _build_nc() -> bass.Bass:
    key = f"nc_f{F}"
    if key in _cache:
        return _cache[key]

    nc = bass.Bass()
    xb_ext = nc.declare_dram_parameter("xb", [KB * P, T], bf16, isOutput=False)
    wb_ext = nc.declare_dram_parameter("wb", [KB * P, D], bf16, isOutput=False)
    b_ext = nc.declare_dram_parameter("bias", [P, OT], f32, isOutput=False)
    out_ext = nc.declare_dram_parameter("out", [D, T], f32, isOutput=True)
    if F:
        x8_ext = nc.declare_dram_parameter("x8", [2 * F * P, T], f8, isOutput=False)
        w8_ext = nc.declare_dram_parameter("w8", [2 * F * P, D], f8, isOutput=False)
        x8_t = x8_ext.rearrange("(k p) t -> p k t", p=P)
        w8_t = w8_ext.rearrange("(k p) (i m) -> p k i m", p=P, m=P)

    wb_t = wb_ext.rearrange("(k p) (i m) -> p k i m", p=P, m=P)
    xb_t = xb_ext.rearrange("(k p) t -> p k t", p=P)

    KC = 2                       # bf16 x planes per chunk
    NCH = KB // KC               # bf16 x chunks
    NL = 4                       # DMA-completion semaphore lanes
    with contextlib.ExitStack() as stack:
        ec = stack.enter_context
        xb_sb = ec(nc.sbuf_tensor("xb_sb", [P, KB, T], bf16))
        wb_sb = [ec(nc.sbuf_tensor(f"wb_sb{j}", [P, KB, P], bf16)) for j in range(W_BUFS)]
        if F:
            x8_sb = ec(nc.sbuf_tensor("x8_sb", [P, 2 * F, T], f8))
            w8_sb = [ec(nc.sbuf_tensor(f"w8_sb{j}", [P, 2 * F, P], f8)) for j in range(W_BUFS)]
        b_sb = ec(nc.sbuf_tensor("b_sb", [P, OT], f32))
        o_sb = [ec(nc.sbuf_tensor(f"o_sb{j}", [P, T], f32)) for j in range(2)]
        ps = [ec(nc.psum_tensor(f"ps{j}", [P, T], f32)) for j in range(2)]
        in_sem = ec(nc.semaphore("in_sem"))
        pe_sem = ec(nc.semaphore("pe_sem"))
        act_sem = ec(nc.semaphore("act_sem"))
        f8sem = ec(nc.semaphore("f8sem"))   # w8 strips 0+1 and x8 (3 DMAs)
        wsem = [ec(nc.semaphore(f"wsem{j}")) for j in range(NL)]
        odsem = [ec(nc.semaphore(f"odsem{j}")) for j in range(NL)]
        xs = [ec(nc.semaphore(f"xs{j}")) for j in range(NCH)]
        block = ec(nc.Block())

        # Per-strip completion bookkeeping: strip i's DMAs increment
        # wsem[i % NL]; with <=3 strips in flight the active strips always
        # sit on distinct lanes, so each threshold is unambiguous.  Strips
        # 0/1 put their (tiny) fp8 part on f8sem instead so the DoubleRow
        # matmuls of o_tiles 0/1 can start before the bf16 strips land.
        lane_tot = [0] * NL
        strip_thr = []
        for i in range(OT):
            inc = 16 if (F == 0 or i < 2) else 32
            lane_tot[i % NL] += inc
            strip_thr.append(lane_tot[i % NL])

        od_tot = [0] * NL
        od_thr = []
        for n in range(OT * NSL):
            od_tot[n % NL] += 16
            od_thr.append(od_tot[n % NL])

        def x_chunk(eng, c):
            eng.dma_start(
                out=xb_sb[:, c * KC:(c + 1) * KC, :],
                in_=xb_t[:, c * KC:(c + 1) * KC, :],
            ).then_inc(xs[c], 16)

        def w_strip(eng, i, buf):
            if F:
                eng.dma_start(out=w8_sb[buf][:], in_=w8_t[:, :, i, :]).then_inc(
                    f8sem if i < 2 else wsem[i % NL], 16)
            eng.dma_start(out=wb_sb[buf][:], in_=wb_t[:, :, i, :]).then_inc(
                wsem[i % NL], 16)

        def wait_strip(eng, i):
            eng.wait_ge(wsem[i % NL], strip_thr[i])
            if F and i < 2:
                eng.wait_ge(f8sem, 48)

        @block.gpsimd
        def _(gp):
            for c in range(1, NCH, 2):
                x_chunk(gp, c)
            # output writeback, one DMA per (o_tile, slice)
            for i in range(OT):
                for s in range(NSL):
                    n = i * NSL + s
                    gp.wait_ge(act_sem, n + 1)
                    gp.dma_start(
                        out=out_ext[i * P:(i + 1) * P, s * NS:(s + 1) * NS],
                        in_=o_sb[i % 2][:, s * NS:(s + 1) * NS],
                    ).then_inc(odsem[n % NL], 16)

        @block.sync
        def _(sync):
            # startup: fp8 strips+x8 first (small, unblock DR matmuls), then
            # the bf16 strips for o_tiles 0/1, then join the x chunk stream.
            if F:
                sync.dma_start(out=w8_sb[0][:], in_=w8_t[:, :, 0, :]).then_inc(f8sem, 16)
                sync.dma_start(out=w8_sb[1][:], in_=w8_t[:, :, 1, :]).then_inc(f8sem, 16)
                sync.dma_start(out=x8_sb[:], in_=x8_t[:]).then_inc(f8sem, 16)
            sync.dma_start(out=b_sb[:], in_=b_ext[:]).then_inc(in_sem, 16)
            sync.dma_start(out=wb_sb[0][:], in_=wb_t[:, :, 0, :]).then_inc(wsem[0], 16)
            sync.dma_start(out=wb_sb[1][:], in_=wb_t[:, :, 1, :]).then_inc(wsem[1], 16)
            for c in range(0, NCH, 2):
                x_chunk(sync, c)
            w_strip(sync, 2, 2)
            for i in range(OT - W_BUFS):
                # strip i+3 lands in the buffer o_tile i just vacated
                sync.wait_ge(pe_sem, i + 1)
                w_strip(sync, i + W_BUFS, (i + W_BUFS) % W_BUFS)
            for j in range(NL):
                if od_tot[j]:
                    sync.wait_ge(odsem[j], od_tot[j])

        @block.tensor
        def _(pe):
            def o_mms(i, s):
                """All matmuls for (o_tile i, slice s): F DR + KB bf16."""
                buf = i % W_BUFS if i >= 2 else i
                psl = ps[i % 2][:, s * NS:(s + 1) * NS]
                xsl = slice(s * NS, (s + 1) * NS)
                n = 0
                if F:
                    for j in range(F):
                        pe.matmul(
                            psl,
                            lhsT=w8_sb[buf][:, 2 * j:2 * j + 2, :],
                            rhs=x8_sb[:, 2 * j:2 * j + 2, xsl],
                            start=(n == 0), stop=False, perf_mode=DR,
                        )
                        n += 1
                for kb in range(KB):
                    mm = pe.matmul(
                        psl,
                        lhsT=wb_sb[buf][:, kb, :],
                        rhs=xb_sb[:, kb, xsl],
                        start=(n == 0 and not F), stop=(kb == KB - 1),
                    )
                    n += 1
                return mm

            # o_tiles 0+1 interleaved, chunk-gated: PE consumes each arriving
            # x chunk 4x (2 o_tiles x 2 slices) so the DMA stream stays ahead.
            if F:
                # DR matmuls only need the fp8 strips 0/1 + x8 (f8sem counts
                # exactly those three DMAs), issued ahead of the bf16 strips.
                pe.wait_ge(f8sem, 48)
                for j in range(F):
                    for oi in range(2):
                        for s in range(NSL):
                            pe.matmul(
                                ps[oi][:, s * NS:(s + 1) * NS],
                                lhsT=w8_sb[oi][:, 2 * j:2 * j + 2, :],
                                rhs=x8_sb[:, 2 * j:2 * j + 2, s * NS:(s + 1) * NS],
                                start=(j == 0), stop=False, perf_mode=DR,
                            )
            pe.wait_ge(wsem[0], 16)
            pe.wait_ge(wsem[1], 16)
            for kb in range(KB):
                if kb % KC == 0:
                    pe.wait_ge(xs[kb // KC], 16)
                for oi in range(2):
                    for s in range(NSL):
                        mm = pe.matmul(
                            ps[oi][:, s * NS:(s + 1) * NS],
                            lhsT=wb_sb[oi][:, kb, :],
                            rhs=xb_sb[:, kb, s * NS:(s + 1) * NS],
                            start=(kb == 0 and not F), stop=(kb == KB - 1),
                        )
                        if kb == KB - 1 and s == NSL - 1 and oi == 1:
                            mm.then_inc(pe_sem, 1)

            # o_tiles 2..31 sequential, PSUM double-buffered
            for i in range(2, OT):
                wait_strip(pe, i)
                # wait for the drain of the o_tile that last used this PSUM buf
                pe.wait_ge(act_sem, (i - 2) * NSL + NSL)
                for s in range(NSL):
                    mm = o_mms(i, s)
                mm.then_inc(pe_sem, 1)

        @block.scalar
        def _(act):
            act.wait_ge(in_sem, 16)  # bias loaded
            for i in range(OT):
                # o_tiles 0/1 complete together (pe_sem hits 1 after the
                # interleaved pass); thereafter pe_sem i means o_tile i done.
                act.wait_ge(pe_sem, 1 if i < 2 else i)
                if i >= 2:
                    # o_sb[i % 2] reuse: o_tile i-2 writeback must be done
                    for s in range(NSL):
                        n = (i - 2) * NSL + s
                        act.wait_ge(odsem[n % NL], od_thr[n])
                for s in range(NSL):
                    act.activation(
                        o_sb[i % 2][:, s * NS:(s + 1) * NS],
                        ps[i % 2][:, s * NS:(s + 1) * NS],
                        mybir.ActivationFunctionType.Identity,
                        bias=b_sb[:, i:i + 1],
                        scale=1.0 / PSCALE,
                    ).then_inc(act_sem, 1)

    _cache[key] = nc
    return nc


def _fold_weights(W_inner, A, B, sparse_values, sparse_indices):
    """W_eff = W_inner + fp16rt(A) @ fp16rt(B) + scatter(fp16rt(values))."""
    A16 = A.astype(np.float16).astype(np.float32)
    B16 = B.astype(np.float16).astype(np.float32)
    V16 = sparse_values.astype(np.float16).astype(np.float32)
    W = W_inner + A16 @ B16
    rows = np.asarray(sparse_indices[0], dtype=np.int64)
    cols = np.asarray(sparse_indices[1], dtype=np.int64)
    S = np.bincount(rows * D + cols, weights=V16, minlength=D * D)
    W += S.reshape(D, D).astype(np.float32)
    return W


def _q8(t, s):
    return np.clip(t * s, -240.0, 240.0).astype(ml_dtypes.float8_e4m3)


def build_inmaps(inputs):
    x = np.asarray(inputs["x"], dtype=np.float32)
    W_inner = np.asarray(inputs["W_inner"], dtype=np.float32)
    b_inner = np.asarray(inputs["b_inner"], dtype=np.float32)
    A = np.asarray(inputs["A"], dtype=np.float32)
    B = np.asarray(inputs["B"], dtype=np.float32)
    sparse_values = np.asarray(inputs["sparse_values"], dtype=np.float32)
    sparse_indices = np.asarray(inputs["sparse_indices"])

    W = _fold_weights(W_inner, A, B, sparse_values, sparse_indices)
    wT = np.ascontiguousarray(W.T)                       # [d_in, d_out] f32
    biasT = np.ascontiguousarray(b_inner.reshape(OT, P).T)  # [128, OT]
    x2T = x.reshape(TOKENS, D).T                         # [d_in, tokens] f32

    KF = 2 * F * P  # rows of the k-dim handled in fp8
    w8 = _q8(wT[:KF], SW)                                # [KF, d_out] fp8
    wb = np.ascontiguousarray((wT[KF:] * PSCALE).astype(ml_dtypes.bfloat16))
    x8_full = _q8(x2T[:KF], SX)
    xb_full = x2T[KF:].astype(ml_dtypes.bfloat16)

    in_maps = []
    for c in range(N_CORES):
        sl = slice(c * T, (c + 1) * T)
        m = {
            "xb": np.ascontiguousarray(xb_full[:, sl]),
            "wb": wb,
            "bias": biasT,
        }
        if F:
            m["x8"] = np.ascontiguousarray(x8_full[:, sl])
            m["w8"] = w8
        in_maps.append(m)
    return in_maps


def run_device(in_maps, **kwargs):
    nc = _build_nc()
    return run_bass_kernel_spmd(nc, in_maps, core_ids=list(range(N_CORES)), **kwargs)


def postprocess(results, dtype=np.float32):
    out = np.empty((TOKENS, D), dtype=dtype)
    for c in range(N_CORES):
        out[c * T:(c + 1) * T, :] = results[c]["out"].T
    return out.reshape(B_SZ, S_SZ, D)


def kernel(**inputs) -> np.ndarray:
    in_maps = build_inmaps(inputs)
    res = run_device(in_maps)
    return postprocess(res.results, dtype=np.asarray(inputs["x"]).dtype)
